# revision 32
# baseline (speedup 1.0000x reference)
"""Barycentric pooling (segmented Sinkhorn onto a 16x256 codebook) on 8 trn2 cores.

Data-parallel: 16 graphs per core, graph g owns partitions [8g, 8g+8).
Per-graph nodes padded to Q (mult of 32); per-partition point count = Q.

Algebraic restructure vs the reference:
  * K is stored per-point-scaled: K' = exp(20*x.c - 10*yn - C) = K * exp(10*xn - C).
    A per-point positive rescale of K commutes through Sinkhorn (u absorbs it)
    and cancels exactly in the final row-normalized weights, so the xn term and
    its matmul chunk are dropped.  C = 20*max|x|*max|c| keeps the exp arg <= 0.
  * a_g is folded into the graph-sum selection matrix (selp), not into u.
  * reciprocals run on VectorE (reciprocal_approx_fast); ScalarE only does the
    phase-1 exp so no activation-table reloads occur mid-iteration.

K layout: q-major [128, Q, 16] bf16 straight from the xbar transpose.
Per iteration (all bulk work = plain tensor_tensor at 2x + tree folds):
  pass 1: prod = K * vrep (broadcast over q), tree-fold j -> d
  u = 1/(d+1e-8) on VectorE; u duplicated into (u,u) bf16 pairs
  pass 2: W = K * u_pairs (pair-broadcast keeps innermost stride 1), tree-fold q
  sraw = selp^T res (TensorE), v = (1/16)/(sraw+1e-8) (VectorE, tiny)
Phase 1: 512-pt tiles, 2 contraction chunks LDW-batched in groups of 4,
exp-ACT per tile into a round-sized kq, one [16,4096] xbar transpose per round
directly into K.
"""

import numpy as np
import ml_dtypes

import concourse.bass as bass
import concourse.bacc as bacc
import concourse.mybir as mybir
from concourse import tile
from concourse.bass_utils import run_bass_kernel_spmd

B = 128          # graphs
CB = 16          # codebook size
HID = 256
DIST = 8
EPS = 0.1
ITERS = 20
NCORES = 8
GPC = B // NCORES  # graphs per core = 16

F32 = mybir.dt.float32
BF16 = mybir.dt.bfloat16
MULT = mybir.AluOpType.mult
ADD = mybir.AluOpType.add
EXPF = mybir.ActivationFunctionType.Exp


def build_nc(Q: int):
    """Q = padded nodes per graph = points per partition; Q % 32 == 0."""
    NPTS = 128 * Q
    n_tiles = NPTS // 512
    n_rounds = n_tiles // 8
    assert n_rounds * 8 == n_tiles

    nc = bacc.Bacc(target_bir_lowering=False, debug=False)

    xt_ext = nc.declare_dram_parameter("xt", [HID, NPTS], BF16, isOutput=False)
    wm_ext = nc.declare_dram_parameter("wmat", [HID, CB], BF16, isOutput=False)
    ynb_ext = nc.declare_dram_parameter("ynb", [CB, 1], F32, isOutput=False)
    selp_ext = nc.declare_dram_parameter("selp", [128, CB], BF16, isOutput=False)
    selr_ext = nc.declare_dram_parameter("selr", [128, 128], BF16, isOutput=False)
    out_ext = nc.declare_dram_parameter("out", [CB, CB], F32, isOutput=True)

    with tile.TileContext(nc) as tc:
        with (
            tc.tile_pool(name="const", bufs=1) as cpool,
            tc.tile_pool(name="stage", bufs=2) as spool,
            tc.tile_pool(name="kq", bufs=2) as kqpool,
            tc.tile_pool(name="work", bufs=2) as wpool,
            tc.tile_pool(name="small", bufs=2) as mpool,
            tc.tile_pool(name="acc", bufs=6, space=bass.MemorySpace.PSUM) as apool,
            tc.tile_pool(name="psmall", bufs=1, space=bass.MemorySpace.PSUM) as ppool,
        ):
            # ---- constants ----
            wm_sb = cpool.tile([128, 2, CB], BF16, tag="wm")
            nc.sync.dma_start(wm_sb[:, 0, :], wm_ext[0:128, :])
            nc.sync.dma_start(wm_sb[:, 1, :], wm_ext[128:256, :])
            ynb_sb = cpool.tile([CB, 1], F32, tag="ynb")
            nc.sync.dma_start(ynb_sb[:], ynb_ext[:, :])
            selp_sb = cpool.tile([128, CB], BF16, tag="selp")
            nc.sync.dma_start(selp_sb[:], selp_ext[:, :])
            selr_sb = cpool.tile([128, 128], BF16, tag="selr")
            nc.sync.dma_start(selr_sb[:], selr_ext[:, :])

            k_qm = cpool.tile([128, Q, CB], BF16, tag="K")
            # replicated v (v[g(p), j] at partition p); starts at ones
            vrep_bf = cpool.tile([128, CB], BF16, tag="vrep")
            nc.vector.memset(vrep_bf[:], 1.0)

            # ---- phase 1: K' = exp(20*x.c - 10*yn - C), q-major ----
            for r in range(n_rounds):
                cols = slice(r * 4096, (r + 1) * 4096)
                xa0 = spool.tile([128, 4096], BF16, tag="xa0")
                xa1 = spool.tile([128, 4096], BF16, tag="xa1")
                nc.sync.dma_start(xa0[:], xt_ext[0:128, cols])
                nc.scalar.dma_start(xa1[:], xt_ext[128:256, cols])
                kq = kqpool.tile([CB, 4096], BF16, tag="kq")
                for sub in range(2):
                    sl = [slice((4 * sub + i) * 512, (4 * sub + i + 1) * 512)
                          for i in range(4)]
                    sa = [apool.tile([CB, 512], F32, tag="acc", name="acc")
                          for _ in range(4)]
                    for i in range(4):
                        nc.tensor.matmul(sa[i][:], wm_sb[:, 0, :],
                                         xa0[:, sl[i]], start=True, stop=False)
                    for i in range(4):
                        nc.tensor.matmul(sa[i][:], wm_sb[:, 1, :],
                                         xa1[:, sl[i]], start=False, stop=True)
                    for i in range(4):
                        nc.scalar.activation(kq[:, sl[i]], sa[i][:], EXPF,
                                             bias=ynb_sb[:], scale=20.0)
                # one xbar transpose per round straight into K
                nc.sync.dma_start_transpose(
                    k_qm[:, 32 * r:32 * r + 32, :], kq[:])

            # ---- phase 2: 20 Sinkhorn iterations ----
            for it in range(ITERS):
                # pass 1: prod = K * vrep (bcast over q), tree-fold j
                prod = wpool.tile([128, Q, CB], BF16, tag="prod")
                if it == 0:
                    # v = 1: skip the multiply, fold K directly (level 1
                    # writes prod so K stays intact)
                    nc.vector.tensor_tensor(prod[:, :, 0:8], k_qm[:, :, 0:8],
                                            k_qm[:, :, 8:16], ADD)
                    n = 8
                else:
                    # j-halves: half h's multiply only needs vrep[:, h-half],
                    # letting half B's v-update chain hide under half A's ops
                    for jh in range(2):
                        sl = slice(8 * jh, 8 * jh + 8)
                        nc.vector.tensor_tensor(
                            prod[:, :, sl], k_qm[:, :, sl],
                            vrep_bf[:, sl].unsqueeze(1).broadcast_to(
                                [128, Q, 8]), MULT)
                    n = CB
                while n > 2:
                    k = n // 2
                    nc.vector.tensor_tensor(prod[:, :, 0:k], prod[:, :, 0:k],
                                            prod[:, :, n - k:n], ADD)
                    n = n - k
                # last fold level fused with the +1e-8 (strided last level is
                # slow as a plain TT; STT does both in one pass)
                d32 = mpool.tile([128, Q], F32, tag="d32")
                nc.vector.scalar_tensor_tensor(
                    d32[:], prod[:, :, 0], 1e-8, prod[:, :, 1], ADD, ADD)
                u32 = mpool.tile([128, Q], F32, tag="u32")
                nc.vector.reciprocal_approx_fast(u32[:], d32[:])
                u2 = mpool.tile([128, Q, 2], BF16, tag="u2")
                nc.vector.tensor_copy(u2[:, :, 0], u32[:])
                nc.vector.tensor_copy(u2[:, :, 1], u32[:])
                # pass 2: W = K * u (pair-bcast), tree-fold q
                w2 = wpool.tile([128, Q, CB], BF16, tag="prod")
                nc.vector.tensor_tensor(
                    w2[:].rearrange("p q (a b) -> p q a b", b=2),
                    k_qm[:].rearrange("p q (a b) -> p q a b", b=2),
                    u2[:].unsqueeze(2).broadcast_to([128, Q, 8, 2]), MULT)
                n = Q
                while n > 4:
                    k = n // 2
                    nc.vector.tensor_tensor(w2[:, 0:k, :], w2[:, 0:k, :],
                                            w2[:, n - k:n, :], ADD)
                    n = n - k
                if it < ITERS - 1:
                    srep = ppool.tile([128, CB], F32, tag="srep")
                    sv32 = mpool.tile([128, CB], F32, tag="sv32")
                    rv = mpool.tile([128, CB], F32, tag="rv")
                    for jh in range(2):
                        sl = slice(8 * jh, 8 * jh + 8)
                        # finish the q-fold for this j-half only (q: 4 -> 1)
                        nc.vector.tensor_tensor(w2[:, 0:2, sl], w2[:, 0:2, sl],
                                                w2[:, 2:4, sl], ADD)
                        nc.vector.tensor_tensor(w2[:, 0, sl], w2[:, 0, sl],
                                                w2[:, 1, sl], ADD)
                        # srep[p, j] = 16*a*sum_{p' in g(p)} res, replicated
                        nc.tensor.matmul(srep[:, sl], selr_sb[:], w2[:, 0, sl],
                                         start=True, stop=True)
                        # vrep = 1/(16*a*s + 16e-8) = (1/16)/(a*s + 1e-8)
                        nc.vector.tensor_scalar(sv32[:, sl], srep[:, sl],
                                                1.6e-7, None, ADD)
                        nc.vector.reciprocal(rv[:, sl], sv32[:, sl])
                        nc.vector.tensor_copy(vrep_bf[:, sl], rv[:, sl])
                else:
                    # final: graph-major [16,16] path, once
                    nc.vector.tensor_tensor(w2[:, 0:2, :], w2[:, 0:2, :],
                                            w2[:, 2:4, :], ADD)
                    nc.vector.tensor_tensor(w2[:, 0, :], w2[:, 0, :],
                                            w2[:, 1, :], ADD)
                    sraw = ppool.tile([CB, CB], F32, tag="sraw")
                    nc.tensor.matmul(sraw[:], selp_sb[:], w2[:, 0, :],
                                     start=True, stop=True)
                    # v' = 1/(16*a*s + 16e-8); weights = (s*a*v') normalized,
                    # constant factors cancel in the ratio
                    sv32g = mpool.tile([CB, CB], F32, tag="sv32g")
                    nc.vector.tensor_scalar(sv32g[:], sraw[:], 16.0, 1.6e-7,
                                            MULT, ADD)
                    rvg = mpool.tile([CB, CB], F32, tag="rvg")
                    nc.vector.reciprocal(rvg[:], sv32g[:])
                    t2 = mpool.tile([CB, CB], F32, tag="t2")
                    nc.vector.tensor_tensor(t2[:], sraw[:], rvg[:], MULT)
                    den = mpool.tile([CB, 1], F32, tag="den")
                    nc.vector.tensor_reduce(den[:], t2[:],
                                            mybir.AxisListType.X, ADD)
                    nc.vector.tensor_scalar(den[:], den[:], 1e-30, None,
                                            mybir.AluOpType.max)
                    rden = mpool.tile([CB, 1], F32, tag="rden")
                    nc.vector.reciprocal(rden[:], den[:])
                    outw = mpool.tile([CB, CB], F32, tag="outw")
                    nc.vector.tensor_scalar(outw[:], t2[:], rden[:], None,
                                            MULT)
                    nc.sync.dma_start(out_ext[:, :], outw[:])

    return nc


def _host_shard(node_distributions, batch_idx, codebook):
    nd = np.ascontiguousarray(np.asarray(node_distributions, dtype=np.float32))
    bi = np.asarray(batch_idx).astype(np.int64).ravel()
    cb = np.asarray(codebook, dtype=np.float32)
    N, S, D = nd.shape
    assert S == DIST and D == HID

    counts = np.bincount(bi, minlength=B)[:B]
    Q = int(np.ceil(max(int(counts.max()), 32) / 32.0) * 32)
    NPTS = 128 * Q

    order = np.argsort(bi, kind="stable")
    slot = np.full((B, Q), -1, dtype=np.int64)
    mask = np.arange(Q)[None, :] < counts[:, None]
    slot[mask] = order

    x2 = nd.reshape(N * S, D)
    xn = np.einsum("ij,ij->i", x2, x2)
    yn = (cb * cb).sum(1)
    Cshift = float(20.0 * np.sqrt(xn.max()) * np.sqrt(yn.max()))

    # column l of a core's xt holds the point at partition p = l % 128,
    # q-slot q = l // 128 (xbar transpose emission order).
    # Partition p holds graph p//8 (core-local), sub-row r = p % 8.
    # Point (p, q): per-graph flat index m = q*8 + r -> node m//8, s = m%8.
    l = np.arange(NPTS)
    p_of_l = l % 128
    q_of_l = l // 128
    g_of_l = p_of_l // 8
    r_of_l = p_of_l % 8
    m = q_of_l * 8 + r_of_l
    nl_of_l = m // S
    s_of_l = m % S

    wmat = np.ascontiguousarray(cb.T).astype(ml_dtypes.bfloat16)   # [256, 16]
    ynb = (-10.0 * yn - Cshift).astype(np.float32).reshape(CB, 1)
    gidx = np.arange(128) // 8
    selt = (gidx[None, :] == np.arange(CB)[:, None]).astype(np.float32)
    same_g = (gidx[:, None] == gidx[None, :]).astype(np.float64)

    in_maps = []
    for c in range(NCORES):
        g_global = c * GPC + g_of_l
        nid = slot[g_global, nl_of_l]
        valid = nid >= 0
        xi = nid * S + s_of_l
        x = x2[np.where(valid, xi, 0), :]
        x[~valid] = 0.0
        xt = np.ascontiguousarray(x.T).astype(ml_dtypes.bfloat16)  # [256, NPTS]
        ccounts = counts[c * GPC:(c + 1) * GPC].astype(np.float64)
        a = np.where(ccounts > 0, 1.0 / np.maximum(ccounts * S, 1), 0.0)
        selp = selt.T * a[gidx][:, None]
        selr = same_g * (16.0 * a)[gidx][:, None]
        in_maps.append({
            "xt": xt,
            "wmat": wmat,
            "ynb": ynb,
            "selp": np.ascontiguousarray(selp).astype(ml_dtypes.bfloat16),
            "selr": np.ascontiguousarray(selr).astype(ml_dtypes.bfloat16),
        })
    return in_maps, Q


def kernel(node_distributions, batch_idx, codebook, _trace=False, _trace_kwargs=None):
    in_maps, Q = _host_shard(node_distributions, batch_idx, codebook)
    nc = build_nc(Q)
    nc.finalize()
    res = run_bass_kernel_spmd(nc, in_maps, list(range(NCORES)),
                               trace=_trace, **(_trace_kwargs or {}))
    out = np.zeros((B, CB), np.float32)
    for c in range(NCORES):
        out[c * GPC:(c + 1) * GPC, :] = res.results[c]["out"]
    kernel._last_exec_time_ns = res.exec_time_ns
    kernel._last_res = res
    return out
